# revision 1
# baseline (speedup 1.0000x reference)
"""Bass/Trainium2 kernel for nn_BigramLM (dense transformer, 8 NeuronCores).

Sharding: cores (2b, 2b+1) both run the transformer body for batch b
(data-parallel over the 4 batches, pair-replicated); the final vocab
projection is split per pair member (V/2 = 16000 columns each), so all
8 cores carry the dominant final-matmul + logits-DMA load.

Layouts on device (per core):
  h      [128, 8, 1024]  residual stream, natural [t, d], fp32
  xnT    [128, 8, 1024]  LN output transposed [d, t], float32r
  qT/kT  DRAM scratch [pair, 128, t], streamed per head-pair
  v      DRAM scratch [s_tile, 128, h, 64]; ones column appended on load
  scores S^T [s, t] per head; softmax = exp (no max-sub) + causal mask,
         denominators via the ones column of V (row 64 of Y^T)
All large matmuls run in float32r (1 cyc/row at free-dim >= 256).
"""

import sys

sys.path.insert(0, "/opt/trn_rl_repo")

import numpy as np

import concourse.bass as bass
import concourse.mybir as mybir
import concourse.tile as tile
from concourse import bacc
from concourse.bass_utils import run_bass_kernel_spmd
from concourse.masks import make_identity

F32 = mybir.dt.float32
F32R = mybir.dt.float32r
AF = mybir.ActivationFunctionType
ALU = mybir.AluOpType

V, D, H, KD, B, T = 32000, 1024, 16, 64, 4, 1024
F = 4 * D
LAYERS = 4
P = 128
NT = T // P            # 8 token tiles
NTC = T // 512         # 2 token chunks (matmul free dim)
ND = D // P            # 8 d tiles
NF = F // P            # 32 f tiles
NPAIR = H // 2         # 8 head pairs
VSH = V // 2           # 16000 vocab columns per core
VC = 500               # vocab chunk (psum free dim)
NVC = VSH // VC        # 32
EPS = 1e-5
SCALE = 1.0 / float(np.sqrt(KD))


def _dram_ap(handle, offset, pattern):
    t = getattr(handle, "tensor", handle)
    offset = offset + getattr(handle, "offset", 0)
    return bass.AP(tensor=t, offset=offset, ap=[list(p) for p in pattern])


def build_program():
    nc = bacc.Bacc("TRN2", target_bir_lowering=False, debug=False, num_devices=8)

    tn = {}
    tn["x_idx"] = nc.dram_tensor("x_idx", [T, 1], mybir.dt.int32, kind="ExternalInput")
    tn["tok_emb"] = nc.dram_tensor("tok_emb", [V, D], F32, kind="ExternalInput")
    tn["pos_emb"] = nc.dram_tensor("pos_emb", [T, D], F32, kind="ExternalInput")
    for nm, shp, dt in (
        ("wq", [H, D, KD], F32R), ("wk", [H, D, KD], F32R), ("wv", [H, D, KD], F32R),
        ("bq", [H, KD], F32), ("bk", [H, KD], F32), ("bv", [H, KD], F32),
        ("wo", [H, KD, KD], F32), ("bo", [H, KD], F32),
        ("w1", [D, F], F32R), ("b1", [F], F32),
        ("w2", [F, D], F32R), ("b2", [D], F32),
        ("ln1_g", [D], F32), ("ln1_b", [D], F32),
        ("ln2_g", [D], F32), ("ln2_b", [D], F32),
        ("lnf_g", [D], F32), ("lnf_b", [D], F32),
        ("wout", [D, VSH], F32R), ("bout", [1, VSH], F32),
    ):
        tn[nm] = nc.dram_tensor(nm, shp, dt, kind="ExternalInput")
    tn["logits"] = nc.dram_tensor("logits", [T, VSH], F32, kind="ExternalOutput")

    with tile.TileContext(nc) as tc:
        _body(nc, tc, tn)
    nc.compile()
    return nc


def _body(nc, tc, tn):
    const = tc.alloc_tile_pool(name="const", bufs=1)
    pers = tc.alloc_tile_pool(name="pers", bufs=1)
    small = tc.alloc_tile_pool(name="small", bufs=3)
    ev_pool = tc.alloc_tile_pool(name="ev_pool", bufs=2)
    dram = tc.alloc_tile_pool(name="dram", bufs=1, space="DRAM")
    ps_work = tc.alloc_tile_pool(name="ps_work", bufs=2, space="PSUM")
    ps_ff = tc.alloc_tile_pool(name="ps_ff", bufs=4, space="PSUM")
    ps_y = tc.alloc_tile_pool(name="ps_y", bufs=2, space="PSUM")
    _static_pools = [const, pers, small, ev_pool, dram, ps_work, ps_ff, ps_y]

    # ---------------- constants ----------------
    ident = const.tile([P, P], F32, tag="ident")
    make_identity(nc, ident)
    eps_t = const.tile([P, 1], F32, tag="eps_t")
    nc.vector.memset(eps_t, EPS)

    bq_sb = const.tile([P, NPAIR], F32, tag="bq_sb")
    bk_sb = const.tile([P, NPAIR], F32, tag="bk_sb")
    b1_sb = const.tile([P, NF], F32, tag="b1_sb")
    nc.sync.dma_start(out=bq_sb, in_=_dram_ap(tn["bq"], 0, [[1, P], [P, NPAIR]]))
    nc.sync.dma_start(out=bk_sb, in_=_dram_ap(tn["bk"], 0, [[1, P], [P, NPAIR]]))
    nc.sync.dma_start(out=b1_sb, in_=_dram_ap(tn["b1"], 0, [[1, P], [P, NF]]))
    bv_bc = const.tile([P, H, KD], F32, tag="bv_bc")
    nc.sync.dma_start(out=bv_bc, in_=_dram_ap(tn["bv"], 0, [[0, P], [KD, H], [1, KD]]))
    b2_bc = const.tile([P, D], F32, tag="b2_bc")
    nc.sync.dma_start(out=b2_bc, in_=_dram_ap(tn["b2"], 0, [[0, P], [1, D]]))
    wo_aug = const.tile([KD + 1, H, KD], F32, tag="wo_aug")
    nc.sync.dma_start(out=wo_aug[0:KD, :, :],
                      in_=_dram_ap(tn["wo"], 0, [[KD, KD], [KD * KD, H], [1, KD]]))
    nc.sync.dma_start(out=wo_aug[KD:KD + 1, :, :],
                      in_=_dram_ap(tn["bo"], 0, [[0, 1], [KD, H], [1, KD]]))
    ones_v = const.tile([P, NT, 2, 1], F32, tag="ones_v")
    nc.vector.memset(ones_v, 1.0)
    # causal 0/1 masks for diagonal-crossing tiles, one per (s_tile - t_chunk)
    # offset delta in {0, 128, 256, 384}: mask[i, j] = 1 iff j - i >= delta
    masks = []
    for mi in range(4):
        mk = const.tile([P, 512], F32, tag=f"mask{mi}", name=f"mask{mi}")
        nc.gpsimd.memset(mk, 1.0)
        nc.gpsimd.affine_select(
            out=mk, in_=mk, compare_op=ALU.is_ge, fill=0.0,
            base=-mi * P, pattern=[[1, 512]], channel_multiplier=-1)
        masks.append(mk)
    ln_gb = {}
    for nm in ("ln1_g", "ln1_b", "ln2_g", "ln2_b", "lnf_g", "lnf_b"):
        t = const.tile([P, ND], F32, tag=nm, name=nm)
        nc.sync.dma_start(out=t, in_=_dram_ap(tn[nm], 0, [[1, P], [P, ND]]))
        ln_gb[nm] = t

    # ---------------- persistent activations ----------------
    h_sb = pers.tile([P, NT, D], F32, tag="h_sb")
    xnT = pers.tile([P, ND, T], F32R, tag="xnT")
    L_sb = pers.tile([H, T], F32, tag="L_sb")
    linv = pers.tile([P, NT, H], F32, tag="linv")

    qT_d = dram.tile([NPAIR, P, T], F32R, tag="qT_d")
    kT_d = dram.tile([NPAIR, P, T], F32R, tag="kT_d")
    v_d = dram.tile([NT, P, H, KD], F32R, tag="v_d")

    # ---------------- embedding ----------------
    for it in range(NT):
        idx_t = small.tile([P, 1], mybir.dt.int32, tag="idx")
        nc.sync.dma_start(out=idx_t, in_=tn["x_idx"][it * P:(it + 1) * P, :])
        nc.gpsimd.indirect_dma_start(
            out=h_sb[:, it, :], out_offset=None, in_=tn["tok_emb"][:, :],
            in_offset=bass.IndirectOffsetOnAxis(ap=idx_t[:, :1], axis=0))
        pos_t = small.tile([P, D], F32, tag="pos", bufs=2)
        nc.sync.dma_start(out=pos_t, in_=tn["pos_emb"][it * P:(it + 1) * P, :])
        nc.vector.tensor_add(out=h_sb[:, it, :], in0=h_sb[:, it, :], in1=pos_t[:])

    # ---------------- helpers ----------------
    def layer_norm_transpose(g_t, b_t):
        """LN(h) -> xnT (transposed [d, t] f32r), gamma/beta applied post-T."""
        for it in range(NT):
            stats = small.tile([P, 2, 6], F32, tag="bnst")
            mv = small.tile([P, 2], F32, tag="bnmv")
            for sg in range(2):
                nc.vector.bn_stats(out=stats[:, sg, :],
                                   in_=h_sb[:, it, sg * 512:(sg + 1) * 512])
            nc.vector.bn_aggr(out=mv, in_=stats)
            rstd = small.tile([P, 1], F32, tag="rstd")
            nc.scalar.activation(out=rstd, in_=mv[:, 1:2], func=AF.Sqrt,
                                 bias=eps_t[:, :], scale=1.0)
            nc.vector.reciprocal(out=rstd, in_=rstd)
            for idd in range(ND):
                xt = small.tile([P, P], F32, tag="xt")
                nc.vector.tensor_scalar(
                    out=xt, in0=h_sb[:, it, idd * P:(idd + 1) * P],
                    scalar1=mv[:, 0:1], scalar2=rstd,
                    op0=ALU.subtract, op1=ALU.mult)
                tp = ps_work.tile([P, P], F32, tag="ps")
                nc.tensor.transpose(out=tp[:], in_=xt[:], identity=ident[:])
                nc.vector.tensor_scalar(
                    out=xnT[:, idd, it * P:(it + 1) * P], in0=tp[:],
                    scalar1=g_t[:, idd:idd + 1], scalar2=b_t[:, idd:idd + 1],
                    op0=ALU.mult, op1=ALU.add)

    # ---------------- transformer layers (tied weights) ----------------
    for _layer in range(LAYERS):
        acc = pers.tile([P, NT, D], F32, tag="acc", name="acc")
        layer_norm_transpose(ln_gb["ln1_g"], ln_gb["ln1_b"])

        # ---- QKV projections -> DRAM scratch ----
        wqk_pool = tc.alloc_tile_pool(name="wqk_pool", bufs=3)
        for which, wh, bsb, dstT in (("q", tn["wq"], bq_sb, qT_d),
                                     ("k", tn["wk"], bk_sb, kT_d)):
            for hp in range(NPAIR):
                wt = wqk_pool.tile([P, ND, 2, KD], F32R, tag="w" + which)
                for hi in range(2):
                    nc.sync.dma_start(out=wt[:, :, hi, :], in_=_dram_ap(
                        wh, (hp * 2 + hi) * D * KD,
                        [[KD, P], [P * KD, ND], [1, KD]]))
                for tcc in range(NTC):
                    pswt = ps_ff.tile([P, 512], F32, tag="ff")
                    for idd in range(ND):
                        nc.tensor.matmul(
                            out=pswt[:], lhsT=wt[:, idd, :, :],
                            rhs=xnT[:, idd, tcc * 512:(tcc + 1) * 512],
                            start=(idd == 0), stop=(idd == ND - 1))
                    st_t = ev_pool.tile([P, 512], F32R, tag="qk_ev")
                    nc.scalar.activation(out=st_t, in_=pswt[:],
                                         func=AF.Identity,
                                         bias=bsb[:, hp:hp + 1], scale=1.0)
                    nc.sync.dma_start(
                        out=dstT[hp, :, tcc * 512:(tcc + 1) * 512], in_=st_t[:])
        for hc in range(2):
            wvt = wqk_pool.tile([P, ND, 8, KD], F32R, tag="wv", bufs=1)
            for hh in range(8):
                nc.sync.dma_start(out=wvt[:, :, hh, :], in_=_dram_ap(
                    tn["wv"], (hc * 8 + hh) * D * KD,
                    [[KD, P], [P * KD, ND], [1, KD]]))
            for it in range(NT):
                psv = ps_ff.tile([P, 512], F32, tag="ff")
                for idd in range(ND):
                    nc.tensor.matmul(
                        out=psv[:], lhsT=xnT[:, idd, it * P:(it + 1) * P],
                        rhs=wvt[:, idd, :, :].rearrange("p a b -> p (a b)"),
                        start=(idd == 0), stop=(idd == ND - 1))
                v_st = ev_pool.tile([P, 8, KD], F32R, tag="v_ev")
                nc.vector.tensor_tensor(
                    out=v_st[:],
                    in0=psv[:].rearrange("p (a b) -> p a b", a=8),
                    in1=bv_bc[:, hc * 8:(hc + 1) * 8, :], op=ALU.add)
                nc.sync.dma_start(out=v_d[it, :, hc * 8:(hc + 1) * 8, :],
                                  in_=v_st[:])
        wqk_pool.release()

        # ---- attention (S^T layout, exp softmax via ones column of V) ----
        kv_pool = tc.alloc_tile_pool(name="kv_pool", bufs=2)
        pt_pool = tc.alloc_tile_pool(name="pt_pool", bufs=3)
        for hp in range(NPAIR):
            qp = kv_pool.tile([P, T], F32R, tag="qp")
            kp = kv_pool.tile([P, T], F32R, tag="kp")
            vp = kv_pool.tile([P, NT, 2, KD + 1], F32R, tag="vp")
            nc.sync.dma_start(out=qp, in_=qT_d[hp, :, :])
            nc.sync.dma_start(out=kp, in_=kT_d[hp, :, :])
            for hi in range(2):
                nc.sync.dma_start(out=vp[:, :, hi, 0:KD], in_=_dram_ap(
                    v_d, (2 * hp + hi) * KD,
                    [[H * KD, P], [P * H * KD, NT], [1, KD]]))
            nc.scalar.activation(out=vp[:, :, :, KD:KD + 1], in_=ones_v[:],
                                 func=AF.Copy)
            for hi in range(2):
                h_ = 2 * hp + hi
                for tcc in range(NTC):
                    y_ps = ps_y.tile([KD + 1, 512], F32, tag="y_ps")
                    n_st = min(NT, (tcc + 1) * 4)
                    for st in range(n_st):
                        s_ps = ps_work.tile([P, 512], F32, tag="ps")
                        nc.tensor.matmul(
                            out=s_ps[:],
                            lhsT=kp[hi * KD:(hi + 1) * KD, st * P:(st + 1) * P],
                            rhs=qp[hi * KD:(hi + 1) * KD, tcc * 512:(tcc + 1) * 512],
                            start=True, stop=True)
                        pt = pt_pool.tile([P, 512], F32R, tag="pt")
                        nc.scalar.activation(out=pt[:], in_=s_ps[:], func=AF.Exp,
                                             scale=SCALE)
                        if st >= 4 * tcc:  # diagonal-crossing: causal mask
                            nc.vector.tensor_tensor(
                                out=pt[:], in0=pt[:],
                                in1=masks[st - 4 * tcc][:], op=ALU.mult)
                        nc.tensor.matmul(
                            out=y_ps[:], lhsT=vp[:, st, hi, :], rhs=pt[:],
                            start=(st == 0), stop=(st == n_st - 1))
                    y_sb = ev_pool.tile([KD + 1, 512], F32, tag="y_sb")
                    nc.scalar.copy(out=y_sb[:], in_=y_ps[:])
                    nc.sync.dma_start(
                        out=L_sb[h_:h_ + 1, tcc * 512:(tcc + 1) * 512],
                        in_=y_sb[KD:KD + 1, :])
                    for it4 in range(4):
                        it = tcc * 4 + it4
                        o_ps = ps_work.tile([P, KD], F32, tag="ps")
                        nc.tensor.matmul(
                            out=o_ps[:], lhsT=y_sb[:, it4 * P:(it4 + 1) * P],
                            rhs=wo_aug[:, h_, :], start=True, stop=True)
                        nc.scalar.copy(out=acc[:, it, h_ * KD:(h_ + 1) * KD],
                                       in_=o_ps[:])
        pt_pool.release()
        kv_pool.release()
        # softmax denominators -> per-partition reciprocals
        for it in range(NT):
            lt_ps = ps_work.tile([P, H], F32, tag="ps")
            nc.tensor.transpose(out=lt_ps[:], in_=L_sb[:, it * P:(it + 1) * P],
                                identity=ident[0:H, 0:H])
            nc.vector.reciprocal(out=linv[:, it, :], in_=lt_ps[:])
        # normalize + residual
        for it in range(NT):
            lap = linv[:, it, :]
            lbc = bass.AP(tensor=lap.tensor, offset=lap.offset,
                          ap=[list(lap.ap[0]), list(lap.ap[-1]), [0, KD]])
            nc.vector.tensor_tensor(
                out=acc[:, it, :].rearrange("p (h k) -> p h k", h=H),
                in0=acc[:, it, :].rearrange("p (h k) -> p h k", h=H),
                in1=lbc, op=ALU.mult)
            nc.vector.tensor_add(out=h_sb[:, it, :], in0=h_sb[:, it, :],
                                 in1=acc[:, it, :])

        layer_norm_transpose(ln_gb["ln2_g"], ln_gb["ln2_b"])

        # ---- FFN ----
        w1_pool = tc.alloc_tile_pool(name="w1_pool", bufs=4)
        w2_pool = tc.alloc_tile_pool(name="w2_pool", bufs=2)
        aT_g = pers.tile([P, 8, T], F32R, tag="acc", name="aT_g")
        for grp in range(4):
            for fi8 in range(8):
                fi = grp * 8 + fi8
                w1t = w1_pool.tile([P, ND, P], F32R, tag="w1")
                nc.sync.dma_start(out=w1t, in_=_dram_ap(
                    tn["w1"], fi * P, [[F, P], [P * F, ND], [1, P]]))
                for tcc in range(NTC):
                    a_ps = ps_work.tile([P, 512], F32, tag="ps")
                    for idd in range(ND):
                        nc.tensor.matmul(
                            out=a_ps[:], lhsT=w1t[:, idd, :],
                            rhs=xnT[:, idd, tcc * 512:(tcc + 1) * 512],
                            start=(idd == 0), stop=(idd == ND - 1))
                    nc.scalar.activation(
                        out=aT_g[:, fi8, tcc * 512:(tcc + 1) * 512], in_=a_ps[:],
                        func=AF.Relu, bias=b1_sb[:, fi:fi + 1], scale=1.0)
            for dc in range(2):
                w2ts = []
                for wh in range(2):
                    w2t = w2_pool.tile([P, 4, 512], F32R, tag="w2")
                    nc.sync.dma_start(out=w2t, in_=_dram_ap(
                        tn["w2"], (grp * 8 + wh * 4) * P * D + dc * 512,
                        [[D, P], [P * D, 4], [1, 512]]))
                    w2ts.append(w2t)
                for tcc in range(NTC):
                    ff_ps = []
                    for _i4 in range(4):
                        ffp = ps_ff.tile([P, 512], F32, tag="ff")
                        ff_ps.append(ffp)
                    for fi8 in range(8):
                        for it4 in range(4):
                            t0 = tcc * 512 + it4 * P
                            nc.tensor.matmul(
                                out=ff_ps[it4][:],
                                lhsT=aT_g[:, fi8, t0:t0 + P],
                                rhs=w2ts[fi8 // 4][:, fi8 % 4, :],
                                start=(fi8 == 0), stop=(fi8 == 7))
                    for it4 in range(4):
                        it = tcc * 4 + it4
                        nc.vector.tensor_add(
                            out=h_sb[:, it, dc * 512:(dc + 1) * 512],
                            in0=h_sb[:, it, dc * 512:(dc + 1) * 512],
                            in1=ff_ps[it4][:])
        w2_pool.release()
        w1_pool.release()
        # + b2
        for it in range(NT):
            nc.vector.tensor_add(out=h_sb[:, it, :], in0=h_sb[:, it, :],
                                 in1=b2_bc[:])

    # ---------------- final LN + vocab projection ----------------
    layer_norm_transpose(ln_gb["lnf_g"], ln_gb["lnf_b"])
    wout_pool = tc.alloc_tile_pool(name="wout_pool", bufs=2)
    lg_pool = tc.alloc_tile_pool(name="lg_pool", bufs=1)
    for vc in range(NVC):
        bout_bc = small.tile([P, VC], F32, tag="bout")
        nc.sync.dma_start(out=bout_bc,
                          in_=_dram_ap(tn["bout"], vc * VC, [[0, P], [1, VC]]))
        wtl = wout_pool.tile([P, ND, VC], F32R, tag="wout")
        nc.sync.dma_start(out=wtl, in_=_dram_ap(
            tn["wout"], vc * VC, [[VSH, P], [P * VSH, ND], [1, VC]]))
        lg_sb = lg_pool.tile([P, NT, VC], F32, tag="lg_sb")
        for it in range(NT):
            lg_ps = ps_ff.tile([P, VC], F32, tag="ff")
            for idd in range(ND):
                nc.tensor.matmul(
                    out=lg_ps[:], lhsT=xnT[:, idd, it * P:(it + 1) * P],
                    rhs=wtl[:, idd, :],
                    start=(idd == 0), stop=(idd == ND - 1))
            nc.vector.tensor_add(out=lg_sb[:, it, :], in0=lg_ps[:],
                                 in1=bout_bc[:])
        nc.sync.dma_start(
            out=_dram_ap(tn["logits"], vc * VC,
                         [[VSH, P], [P * VSH, NT], [1, VC]]),
            in_=lg_sb[:])
    lg_pool.release()
    wout_pool.release()
    for _p in reversed(_static_pools):
        _p.release()


_PROGRAM = None


def _get_program():
    global _PROGRAM
    if _PROGRAM is None:
        _PROGRAM = build_program()
    return _PROGRAM


def make_in_maps(inputs):
    f = lambda k: np.ascontiguousarray(np.asarray(inputs[k], dtype=np.float32))
    x = np.asarray(inputs["x"]).astype(np.int32)          # [B, T]
    shared = {
        "tok_emb": f("tok_emb"), "pos_emb": f("pos_emb"),
        "wq": f("Wq"), "wk": f("Wk"), "wv": f("Wv"),
        "bq": f("bq"), "bk": f("bk"), "bv": f("bv"),
        "wo": f("Wo"), "bo": f("bo"),
        "w1": f("W1"), "b1": f("b1"), "w2": f("W2"), "b2": f("b2"),
        "ln1_g": f("ln1_g"), "ln1_b": f("ln1_b"),
        "ln2_g": f("ln2_g"), "ln2_b": f("ln2_b"),
        "lnf_g": f("lnf_g"), "lnf_b": f("lnf_b"),
    }
    wout_full = f("Wout")
    bout_full = f("bout").reshape(1, V)
    in_maps = []
    for c in range(8):
        b, vh = c // 2, c % 2
        m = dict(shared)
        m["x_idx"] = np.ascontiguousarray(x[b].reshape(T, 1))
        m["wout"] = np.ascontiguousarray(wout_full[:, vh * VSH:(vh + 1) * VSH])
        m["bout"] = np.ascontiguousarray(bout_full[:, vh * VSH:(vh + 1) * VSH])
        in_maps.append(m)
    return in_maps


def kernel(**inputs):
    in_maps = make_in_maps(inputs)
    nc = _get_program()
    res = run_bass_kernel_spmd(nc, in_maps, core_ids=list(range(8)))
    out = np.empty((B, T, V), dtype=np.float32)
    for c in range(8):
        b, vh = c // 2, c % 2
        out[b, :, vh * VSH:(vh + 1) * VSH] = res.results[c]["logits"]
    return out



# revision 2
# speedup vs baseline: 1.1276x; 1.1276x over previous
"""Bass/Trainium2 kernel for nn_BigramLM (dense transformer, 8 NeuronCores), v2.

Sharding: cores (2b, 2b+1) both run the transformer body for batch b
(data-parallel over the 4 batches, pair-replicated); the final vocab
projection is split per pair member (V/2 = 16000 columns each).

v2 changes vs baseline:
  - all matmul operands bf16 (psum stays fp32): fixes the fp32r LDWEIGHTS /
    stream-rate penalties seen in the trace (scores 655ns -> ~220ns etc).
  - LN gamma/beta folded into the following weights/biases host-side, so
    on-device LN is stats + normalize + transpose only.
  - QKV/attention activations stay in SBUF (no DRAM scratch round-trips).
  - scores computed per head-pair with row-group packed matmuls
    (tile_position (0,0)/(64,0)), exp over [128,1024] dual tiles.
  - causal masks applied on GpSimd (otherwise idle), freeing DVE.
  - attention o = concat_h(y_h @ Wo_h) accumulated column-wise in one psum
    bank per (it, half); softmax normalization folded after Wo via the
    denominator row produced by the augmented-V ones column.
  - FFN accumulates all 32 f-tiles into resident psum, evacuating once.
"""

import sys

sys.path.insert(0, "/opt/trn_rl_repo")

import numpy as np

import concourse.bass as bass
import concourse.mybir as mybir
import concourse.tile as tile
from concourse import bacc
from concourse.bass_utils import run_bass_kernel_spmd
from concourse.masks import make_identity

F32 = mybir.dt.float32
BF16 = mybir.dt.bfloat16
AF = mybir.ActivationFunctionType
ALU = mybir.AluOpType

V, D, H, KD, B, T = 32000, 1024, 16, 64, 4, 1024
F = 4 * D
LAYERS = 4
P = 128
NT = T // P            # 8 token tiles
NTC = T // 512         # 2 token chunks (matmul free dim)
ND = D // P            # 8 d tiles
NF = F // P            # 32 f tiles
NPAIR = H // 2         # 8 head pairs
VSH = V // 2           # 16000 vocab columns per core
VC = 500               # vocab chunk (psum free dim)
NVC = VSH // VC        # 32
EPS = 1e-5
SCALE = 1.0 / float(np.sqrt(KD))

PACK_SCORES = True     # row-group packed dual-head score matmuls


def _dram_ap(handle, offset, pattern):
    t = getattr(handle, "tensor", handle)
    offset = offset + getattr(handle, "offset", 0)
    return bass.AP(tensor=t, offset=offset, ap=[list(p) for p in pattern])


def build_program():
    nc = bacc.Bacc("TRN2", target_bir_lowering=False, debug=False, num_devices=8)

    tn = {}
    tn["x_idx"] = nc.dram_tensor("x_idx", [T, 1], mybir.dt.int32, kind="ExternalInput")
    tn["tok_emb"] = nc.dram_tensor("tok_emb", [V, D], F32, kind="ExternalInput")
    tn["pos_emb"] = nc.dram_tensor("pos_emb", [T, D], F32, kind="ExternalInput")
    for nm, shp, dt in (
        ("wq", [H, D, KD], BF16), ("wk", [H, D, KD], BF16), ("wv", [H, D, KD], BF16),
        ("bq", [H, KD], F32), ("bk", [H, KD], F32), ("bv", [H, KD], F32),
        ("wo", [H, KD, KD], BF16), ("bo", [H, KD], BF16),
        ("w1", [D, F], BF16), ("b1", [F], F32),
        ("w2", [F, D], BF16), ("b2", [D], F32),
        ("wout", [D, VSH], BF16), ("bout", [1, VSH], F32),
    ):
        tn[nm] = nc.dram_tensor(nm, shp, dt, kind="ExternalInput")
    tn["logits"] = nc.dram_tensor("logits", [T, VSH], F32, kind="ExternalOutput")

    with tile.TileContext(nc) as tc:
        _body(nc, tc, tn)
    nc.compile()
    return nc


def _body(nc, tc, tn):
    const = tc.alloc_tile_pool(name="const", bufs=1)
    pers_x = tc.alloc_tile_pool(name="pers_x", bufs=1)   # xnT: lives to the end
    small = tc.alloc_tile_pool(name="small", bufs=3)
    ev = tc.alloc_tile_pool(name="ev", bufs=2)
    ps = tc.alloc_tile_pool(name="ps", bufs=1, space="PSUM")
    # top of the SBUF pool stack: released before the vocab projection
    pers = tc.alloc_tile_pool(name="pers", bufs=1)

    # ---------------- constants ----------------
    ident_bf = const.tile([P, P], BF16, tag="ident_bf")
    make_identity(nc, ident_bf)
    eps_t = const.tile([P, 1], F32, tag="eps_t")
    nc.vector.memset(eps_t, EPS)

    bq_sb = const.tile([P, NPAIR], F32, tag="bq_sb")
    bk_sb = const.tile([P, NPAIR], F32, tag="bk_sb")
    b1_sb = const.tile([P, NF], F32, tag="b1_sb")
    nc.sync.dma_start(out=bq_sb, in_=_dram_ap(tn["bq"], 0, [[1, P], [P, NPAIR]]))
    nc.sync.dma_start(out=bk_sb, in_=_dram_ap(tn["bk"], 0, [[1, P], [P, NPAIR]]))
    nc.sync.dma_start(out=b1_sb, in_=_dram_ap(tn["b1"], 0, [[1, P], [P, NF]]))
    bv_bc = const.tile([P, H, KD], F32, tag="bv_bc")
    nc.sync.dma_start(out=bv_bc, in_=_dram_ap(tn["bv"], 0, [[0, P], [KD, H], [1, KD]]))
    b2_bc = const.tile([P, D], F32, tag="b2_bc")
    nc.sync.dma_start(out=b2_bc, in_=_dram_ap(tn["b2"], 0, [[0, P], [1, D]]))
    wo_aug = const.tile([KD + 1, H, KD], BF16, tag="wo_aug")
    nc.sync.dma_start(out=wo_aug[0:KD, :, :],
                      in_=_dram_ap(tn["wo"], 0, [[KD, KD], [KD * KD, H], [1, KD]]))
    nc.sync.dma_start(out=wo_aug[KD:KD + 1, :, :],
                      in_=_dram_ap(tn["bo"], 0, [[0, 1], [KD, H], [1, KD]]))
    # dual-head causal masks: masks[mi][:, half*512 + j] = 1 iff j - p >= mi*128
    masks = []
    for mi in range(4):
        mk = const.tile([P, 2, 512], BF16, tag=f"mask{mi}", name=f"mask{mi}")
        nc.gpsimd.memset(mk, 1.0)
        for half in range(2):
            nc.gpsimd.affine_select(
                out=mk[:, half, :], in_=mk[:, half, :], compare_op=ALU.is_ge,
                fill=0.0, base=-mi * P, pattern=[[1, 512]], channel_multiplier=-1)
        masks.append(mk)

    # ---------------- persistent activations ----------------
    h_sb = pers.tile([P, NT, D], F32, tag="h_sb")
    xnT = pers_x.tile([P, ND, T], BF16, tag="xnT")
    qT = pers.tile([P, NPAIR, T], BF16, tag="qT")
    kT = pers.tile([P, NPAIR, T], BF16, tag="kT")
    v_aug = pers.tile([P, NT, H, KD + 1], BF16, tag="v_aug")
    # scratch doubles as y16 (attention, [65, H, 512] slice) and aT_g (FFN);
    # the two uses are phase-disjoint within a layer.
    scratch = pers.tile([P, NF, 512], BF16, tag="scratch")
    L_sb = pers.tile([H, T], BF16, tag="L_sb")
    linv = pers.tile([P, NT, H], F32, tag="linv")

    nc.vector.memset(v_aug[:, :, :, KD:KD + 1], 1.0)

    # ---------------- embedding ----------------
    for it in range(NT):
        idx_t = small.tile([P, 1], mybir.dt.int32, tag="idx")
        nc.sync.dma_start(out=idx_t, in_=tn["x_idx"][it * P:(it + 1) * P, :])
        nc.gpsimd.indirect_dma_start(
            out=h_sb[:, it, :], out_offset=None, in_=tn["tok_emb"][:, :],
            in_offset=bass.IndirectOffsetOnAxis(ap=idx_t[:, :1], axis=0))
        pos_t = small.tile([P, D], F32, tag="pos", bufs=1)
        nc.sync.dma_start(out=pos_t, in_=tn["pos_emb"][it * P:(it + 1) * P, :])
        nc.vector.tensor_add(out=h_sb[:, it, :], in0=h_sb[:, it, :], in1=pos_t[:])

    # ---------------- helpers ----------------
    def layer_norm_T():
        """xnT = transpose(normalize(h_sb)) in bf16 (gamma/beta pre-folded)."""
        for it in range(NT):
            stats = small.tile([P, 2, 6], F32, tag="bnst")
            mv = small.tile([P, 2], F32, tag="bnmv")
            for sg in range(2):
                nc.vector.bn_stats(out=stats[:, sg, :],
                                   in_=h_sb[:, it, sg * 512:(sg + 1) * 512])
            nc.vector.bn_aggr(out=mv, in_=stats)
            rstd = small.tile([P, 1], F32, tag="rstd")
            nc.scalar.activation(out=rstd, in_=mv[:, 1:2], func=AF.Sqrt,
                                 bias=eps_t[:, :], scale=1.0)
            nc.vector.reciprocal(out=rstd, in_=rstd)
            for half in range(2):
                xt4 = ev.tile([P, 512], BF16, tag="xt", bufs=3)
                nc.vector.tensor_scalar(
                    out=xt4, in0=h_sb[:, it, half * 512:(half + 1) * 512],
                    scalar1=mv[:, 0:1], scalar2=rstd,
                    op0=ALU.subtract, op1=ALU.mult)
                tp4 = ps.tile([P, 512], BF16, tag="work", bufs=2)
                for k in range(4):
                    nc.tensor.transpose(out=tp4[:, k * P:(k + 1) * P],
                                        in_=xt4[:, k * P:(k + 1) * P],
                                        identity=ident_bf[:])
                dst = xnT[:, half * 4:(half + 1) * 4, it * P:(it + 1) * P]
                src4 = tp4[:].rearrange("p (a b) -> p a b", a=4)
                if (it + half) % 2 == 0:
                    nc.scalar.copy(out=dst, in_=src4)
                else:
                    nc.vector.tensor_copy(dst, src4)

    # ---------------- transformer layers (tied weights) ----------------
    for _layer in range(LAYERS):
        layer_norm_T()

        # ---- QKV projections (SBUF resident) ----
        for hp in range(NPAIR):
            for which, wh, bsb, dstT in (("q", tn["wq"], bq_sb, qT),
                                         ("k", tn["wk"], bk_sb, kT)):
                wt = small.tile([P, ND, 2, KD], BF16, tag="w" + which, bufs=3)
                for hi in range(2):
                    nc.sync.dma_start(out=wt[:, :, hi, :], in_=_dram_ap(
                        wh, (hp * 2 + hi) * D * KD,
                        [[KD, P], [P * KD, ND], [1, KD]]))
                for tcc in range(NTC):
                    pswt = ps.tile([P, 512], F32, tag="work", bufs=2)
                    for idd in range(ND):
                        nc.tensor.matmul(
                            out=pswt[:], lhsT=wt[:, idd, :, :],
                            rhs=xnT[:, idd, tcc * 512:(tcc + 1) * 512],
                            start=(idd == 0), stop=(idd == ND - 1))
                    nc.scalar.activation(
                        out=dstT[:, hp, tcc * 512:(tcc + 1) * 512], in_=pswt[:],
                        func=AF.Identity, bias=bsb[:, hp:hp + 1], scale=1.0)
        for hc in range(2):
            wvt = small.tile([P, ND, 8, KD], BF16, tag="wv", bufs=1)
            for hh in range(8):
                nc.sync.dma_start(out=wvt[:, :, hh, :], in_=_dram_ap(
                    tn["wv"], (hc * 8 + hh) * D * KD,
                    [[KD, P], [P * KD, ND], [1, KD]]))
            for it in range(NT):
                psv = ps.tile([P, 512], F32, tag="work", bufs=2)
                for idd in range(ND):
                    nc.tensor.matmul(
                        out=psv[:], lhsT=xnT[:, idd, it * P:(it + 1) * P],
                        rhs=wvt[:, idd, :, :].rearrange("p a b -> p (a b)"),
                        start=(idd == 0), stop=(idd == ND - 1))
                nc.vector.tensor_tensor(
                    out=v_aug[:, it, hc * 8:(hc + 1) * 8, 0:KD],
                    in0=psv[:].rearrange("p (a b) -> p a b", a=8),
                    in1=bv_bc[:, hc * 8:(hc + 1) * 8, :], op=ALU.add)

        # ---- attention ----
        for tcc in range(NTC):
            n_st = 4 * (tcc + 1)
            for hp in range(NPAIR):
                y_ps = [ps.tile([KD + 1, 512], F32, tag="work", bufs=2,
                                name=f"y_ps{_hi}")
                        for _hi in range(2)]
                pts = {}

                def emit_scores(st, hp=hp, tcc=tcc, pts=pts):
                    s_ps = ps.tile([P, 2, 512], F32, tag="dual", bufs=3,
                                   name="s_ps")
                    for hi in range(2):
                        nc.tensor.matmul(
                            out=s_ps[:, hi, :],
                            lhsT=kT[hi * KD:(hi + 1) * KD, hp, st * P:(st + 1) * P],
                            rhs=qT[hi * KD:(hi + 1) * KD, hp,
                                   tcc * 512:(tcc + 1) * 512],
                            start=True, stop=True,
                            tile_position=((hi * KD, 0) if PACK_SCORES else None))
                    pt = ev.tile([P, 2, 512], BF16, tag="pt", bufs=4, name="pt")
                    nc.scalar.activation(out=pt[:], in_=s_ps[:], func=AF.Exp,
                                         scale=SCALE)
                    delta = st * P - tcc * 512
                    if delta >= 0:
                        nc.vector.tensor_tensor(
                            out=pt[:], in0=pt[:], in1=masks[delta // P][:],
                            op=ALU.mult)
                    pts[st] = pt

                def emit_av(st, hp=hp, n_st=n_st, y_ps=y_ps, pts=pts):
                    pt = pts.pop(st)
                    for hi in range(2):
                        nc.tensor.matmul(
                            out=y_ps[hi][:],
                            lhsT=v_aug[:, st, 2 * hp + hi, :],
                            rhs=pt[:, hi, :],
                            start=(st == 0), stop=(st == n_st - 1))

                for st in range(n_st):
                    emit_scores(st)
                    if st >= 2:
                        emit_av(st - 2)
                emit_av(n_st - 2)
                emit_av(n_st - 1)
                for hi in range(2):
                    h_ = 2 * hp + hi
                    nc.vector.tensor_copy(scratch[0:KD + 1, h_, :], y_ps[hi][:])
                    nc.sync.dma_start(
                        out=L_sb[h_:h_ + 1, tcc * 512:(tcc + 1) * 512],
                        in_=scratch[KD:KD + 1, h_, :])
            # denominators -> per-token reciprocals
            for it4 in range(4):
                it = tcc * 4 + it4
                lt_ps = ps.tile([P, H], BF16, tag="work", bufs=2)
                nc.tensor.transpose(out=lt_ps[:], in_=L_sb[:, it * P:(it + 1) * P],
                                    identity=ident_bf[0:H, 0:H])
                nc.vector.reciprocal(out=linv[:, it, :], in_=lt_ps[:])
            # o = concat_h(y_h @ Wo_h); h += o * linv (+ bo via ones row)
            for it4 in range(4):
                it = tcc * 4 + it4
                for half in range(2):
                    o_ps = ps.tile([P, 512], F32, tag="work", bufs=2)
                    for h8 in range(8):
                        h_ = half * 8 + h8
                        nc.tensor.matmul(
                            out=o_ps[:, h8 * KD:(h8 + 1) * KD],
                            lhsT=scratch[0:KD + 1, h_, it4 * P:(it4 + 1) * P],
                            rhs=wo_aug[:, h_, :],
                            start=(h8 == 0), stop=(h8 == 7))
                    lap = linv[:, it, half * 8:(half + 1) * 8]
                    lbc = bass.AP(tensor=lap.tensor, offset=lap.offset,
                                  ap=[list(lap.ap[0]), list(lap.ap[-1]), [0, KD]])
                    o_sb = ev.tile([P, 8, KD], F32, tag="o_sb", bufs=3)
                    nc.vector.tensor_tensor(
                        out=o_sb[:], in0=o_ps[:].rearrange("p (a b) -> p a b", a=8),
                        in1=lbc, op=ALU.mult)
                    nc.vector.tensor_tensor(
                        out=h_sb[:, it, half * 512:(half + 1) * 512],
                        in0=h_sb[:, it, half * 512:(half + 1) * 512],
                        in1=o_sb[:].rearrange("p a b -> p (a b)"), op=ALU.add)

        layer_norm_T()

        # ---- FFN ----
        aT_g = scratch
        for tcc in range(NTC):
            for fi in range(NF):
                w1t = small.tile([P, ND, P], BF16, tag="w1", bufs=4)
                nc.sync.dma_start(out=w1t, in_=_dram_ap(
                    tn["w1"], fi * P, [[F, P], [P * F, ND], [1, P]]))
                a_ps = ps.tile([P, 512], F32, tag="work", bufs=2)
                for idd in range(ND):
                    nc.tensor.matmul(
                        out=a_ps[:], lhsT=w1t[:, idd, :],
                        rhs=xnT[:, idd, tcc * 512:(tcc + 1) * 512],
                        start=(idd == 0), stop=(idd == ND - 1))
                nc.scalar.activation(
                    out=aT_g[:, fi, :], in_=a_ps[:],
                    func=AF.Relu, bias=b1_sb[:, fi:fi + 1], scale=1.0)
            for dc in range(2):
                ffp = [ps.tile([P, 2, 512], F32, tag="dual", bufs=3,
                               name=f"ffp{_pp}")
                       for _pp in range(2)]
                for fi in range(NF):
                    w2t = small.tile([P, 512], BF16, tag="w2", bufs=4)
                    nc.sync.dma_start(out=w2t, in_=_dram_ap(
                        tn["w2"], fi * P * D + dc * 512, [[D, P], [1, 512]]))
                    for pair in range(2):
                        for ih in range(2):
                            nc.tensor.matmul(
                                out=ffp[pair][:, ih, :],
                                lhsT=aT_g[:, fi, (pair * 2 + ih) * P:
                                          (pair * 2 + ih + 1) * P],
                                rhs=w2t[:],
                                start=(fi == 0), stop=(fi == NF - 1))
                for pair in range(2):
                    for ih in range(2):
                        it = tcc * 4 + pair * 2 + ih
                        nc.vector.tensor_tensor(
                            out=h_sb[:, it, dc * 512:(dc + 1) * 512],
                            in0=h_sb[:, it, dc * 512:(dc + 1) * 512],
                            in1=ffp[pair][:, ih, :], op=ALU.add)
        for it in range(NT):
            nc.vector.tensor_add(out=h_sb[:, it, :], in0=h_sb[:, it, :],
                                 in1=b2_bc[:])

    # ---------------- final LN + vocab projection ----------------
    layer_norm_T()
    pers.release()
    wout_pool = tc.alloc_tile_pool(name="wout_pool", bufs=3)
    lg_pool = tc.alloc_tile_pool(name="lg_pool", bufs=6)
    for vc in range(NVC):
        bout_bc = small.tile([P, VC], F32, tag="bout", bufs=2)
        nc.sync.dma_start(out=bout_bc,
                          in_=_dram_ap(tn["bout"], vc * VC, [[0, P], [1, VC]]))
        wtl = wout_pool.tile([P, ND, VC], BF16, tag="wout")
        nc.sync.dma_start(out=wtl, in_=_dram_ap(
            tn["wout"], vc * VC, [[VSH, P], [P * VSH, ND], [1, VC]]))
        for it in range(NT):
            lg_ps = ps.tile([P, VC], F32, tag="work", bufs=2)
            for idd in range(ND):
                nc.tensor.matmul(
                    out=lg_ps[:], lhsT=xnT[:, idd, it * P:(it + 1) * P],
                    rhs=wtl[:, idd, :],
                    start=(idd == 0), stop=(idd == ND - 1))
            lg_sb = lg_pool.tile([P, VC], F32, tag="lg")
            nc.vector.tensor_tensor(out=lg_sb[:], in0=lg_ps[:], in1=bout_bc[:],
                                    op=ALU.add)
            nc.sync.dma_start(
                out=_dram_ap(tn["logits"], it * P * VSH + vc * VC,
                             [[VSH, P], [1, VC]]),
                in_=lg_sb[:])
    lg_pool.release()
    wout_pool.release()
    ps.release()
    ev.release()
    small.release()
    pers_x.release()
    const.release()


_PROGRAM = None


def _get_program():
    global _PROGRAM
    if _PROGRAM is None:
        _PROGRAM = build_program()
    return _PROGRAM


def make_in_maps(inputs):
    bf16 = mybir.dt.np(BF16)
    f = lambda k: np.asarray(inputs[k], dtype=np.float32)
    x = np.asarray(inputs["x"]).astype(np.int32)          # [B, T]

    ln1_g, ln1_b = f("ln1_g"), f("ln1_b")
    ln2_g, ln2_b = f("ln2_g"), f("ln2_b")
    lnf_g, lnf_b = f("lnf_g"), f("lnf_b")
    Wq, Wk, Wv = f("Wq"), f("Wk"), f("Wv")
    W1, Wout = f("W1"), f("Wout")

    # fold LN gamma into the consuming weights, beta into the biases
    wq_f = Wq * ln1_g[None, :, None]
    wk_f = Wk * ln1_g[None, :, None]
    wv_f = Wv * ln1_g[None, :, None]
    bq_f = f("bq") + np.einsum("d,hdk->hk", ln1_b, Wq)
    bk_f = f("bk") + np.einsum("d,hdk->hk", ln1_b, Wk)
    bv_f = f("bv") + np.einsum("d,hdk->hk", ln1_b, Wv)
    w1_f = W1 * ln2_g[:, None]
    b1_f = f("b1") + ln2_b @ W1
    wout_f = Wout * lnf_g[:, None]
    bout_f = (f("bout") + lnf_b @ Wout).reshape(1, V)

    c = np.ascontiguousarray
    shared = {
        "tok_emb": c(f("tok_emb")), "pos_emb": c(f("pos_emb")),
        "wq": c(wq_f.astype(bf16)), "wk": c(wk_f.astype(bf16)),
        "wv": c(wv_f.astype(bf16)),
        "bq": c(bq_f), "bk": c(bk_f), "bv": c(bv_f),
        "wo": c(f("Wo").astype(bf16)), "bo": c(f("bo").astype(bf16)),
        "w1": c(w1_f.astype(bf16)), "b1": c(b1_f),
        "w2": c(f("W2").astype(bf16)), "b2": c(f("b2")),
    }
    in_maps = []
    for cix in range(8):
        b, vh = cix // 2, cix % 2
        m = dict(shared)
        m["x_idx"] = c(x[b].reshape(T, 1))
        m["wout"] = c(wout_f[:, vh * VSH:(vh + 1) * VSH].astype(bf16))
        m["bout"] = c(bout_f[:, vh * VSH:(vh + 1) * VSH])
        in_maps.append(m)
    return in_maps


def kernel(**inputs):
    in_maps = make_in_maps(inputs)
    nc = _get_program()
    res = run_bass_kernel_spmd(nc, in_maps, core_ids=list(range(8)))
    out = np.empty((B, T, V), dtype=np.float32)
    for cix in range(8):
        b, vh = cix // 2, cix % 2
        out[b, :, vh * VSH:(vh + 1) * VSH] = res.results[cix]["logits"]
    return out


# revision 3
# speedup vs baseline: 1.2004x; 1.0646x over previous
"""Bass/Trainium2 kernel for nn_BigramLM (dense transformer, 8 NeuronCores), v2.

Sharding v4: sequence-parallel pairs. Core (2b + r) handles tokens
[r*512, (r+1)*512) of batch b for ALL heads/features: embeddings, LN, QKV,
FFN and the vocab projection (full V) are per-token and shard cleanly; for
attention each core produces k/v for its tokens, a per-layer in-pair
AllGather (2MB bf16 bounce via DRAM) assembles the full-T k/v, and causal
masks (host-computed per core, applied to every key tile) handle the
geometry so the SPMD program is identical on both pair members.

v2 changes vs baseline:
  - all matmul operands bf16 (psum stays fp32): fixes the fp32r LDWEIGHTS /
    stream-rate penalties seen in the trace (scores 655ns -> ~220ns etc).
  - LN gamma/beta folded into the following weights/biases host-side, so
    on-device LN is stats + normalize + transpose only.
  - QKV/attention activations stay in SBUF (no DRAM scratch round-trips).
  - scores computed per head-pair with row-group packed matmuls
    (tile_position (0,0)/(64,0)), exp over [128,1024] dual tiles.
  - causal masks applied on GpSimd (otherwise idle), freeing DVE.
  - attention o = concat_h(y_h @ Wo_h) accumulated column-wise in one psum
    bank per (it, half); softmax normalization folded after Wo via the
    denominator row produced by the augmented-V ones column.
  - FFN accumulates all 32 f-tiles into resident psum, evacuating once.
"""

import sys

sys.path.insert(0, "/opt/trn_rl_repo")

import numpy as np

import concourse.bass as bass
import concourse.mybir as mybir
import concourse.tile as tile
from concourse import bacc
from concourse.bass_utils import run_bass_kernel_spmd
from concourse.masks import make_identity

F32 = mybir.dt.float32
BF16 = mybir.dt.bfloat16
AF = mybir.ActivationFunctionType
ALU = mybir.AluOpType

V, D, H, KD, B, T = 32000, 1024, 16, 64, 4, 1024
F = 4 * D
LAYERS = 4
P = 128
TL = 512               # local tokens per core
NT = T // P            # 8 key tiles (full T)
NTL = TL // P          # 4 local token tiles
ND = D // P            # 8 d tiles
NF = F // P            # 32 f tiles
NPAIR = H // 2         # 8 head pairs
KVW = NPAIR * TL + NTL * H * KD   # 8192: flat k|v bounce width per core
VC = 500               # vocab chunk (psum free dim)
NVC = V // VC          # 64 (full vocab per core)
EPS = 1e-5
SCALE = 1.0 / float(np.sqrt(KD))

PACK_SCORES = True     # row-group packed dual-head score matmuls


def _dram_ap(handle, offset, pattern):
    t = getattr(handle, "tensor", handle)
    offset = offset + getattr(handle, "offset", 0)
    return bass.AP(tensor=t, offset=offset, ap=[list(p) for p in pattern])


def build_program():
    nc = bacc.Bacc("TRN2", target_bir_lowering=False, debug=False, num_devices=8)

    tn = {}
    tn["x_idx"] = nc.dram_tensor("x_idx", [TL, 1], mybir.dt.int32, kind="ExternalInput")
    tn["tok_emb"] = nc.dram_tensor("tok_emb", [V, D], F32, kind="ExternalInput")
    tn["pos_emb"] = nc.dram_tensor("pos_emb", [TL, D], F32, kind="ExternalInput")
    tn["cmask"] = nc.dram_tensor("cmask", [NT, P, 2, TL], BF16, kind="ExternalInput")
    for nm, shp, dt in (
        ("wq", [H, D, KD], BF16), ("wk", [H, D, KD], BF16), ("wv", [H, D, KD], BF16),
        ("bq", [H, KD], F32), ("bk", [H, KD], F32), ("bv", [H, KD], F32),
        ("wo", [H, KD, KD], BF16), ("bo", [H, KD], BF16),
        ("w1", [D, F], BF16), ("b1", [F], F32),
        ("w2", [F, D], BF16), ("b2", [D], F32),
        ("wout", [D, V], BF16), ("bout", [1, V], F32),
    ):
        tn[nm] = nc.dram_tensor(nm, shp, dt, kind="ExternalInput")
    tn["logits"] = nc.dram_tensor("logits", [TL, V], F32, kind="ExternalOutput")

    with tile.TileContext(nc) as tc:
        _body(nc, tc, tn)
    nc.compile()
    return nc


GROUPS = [[0, 1], [2, 3], [4, 5], [6, 7]]


def _body(nc, tc, tn):
    const = tc.alloc_tile_pool(name="const", bufs=1)
    pers_x = tc.alloc_tile_pool(name="pers_x", bufs=1)   # xnT: lives to the end
    small = tc.alloc_tile_pool(name="small", bufs=3)
    ev = tc.alloc_tile_pool(name="ev", bufs=2)
    ps = tc.alloc_tile_pool(name="ps", bufs=1, space="PSUM")
    dram = tc.alloc_tile_pool(name="dram", bufs=2, space="DRAM")
    # top of the SBUF pool stack: released before the vocab projection
    pers = tc.alloc_tile_pool(name="pers", bufs=1)

    # ---------------- constants ----------------
    ident_bf = const.tile([P, P], BF16, tag="ident_bf")
    make_identity(nc, ident_bf)
    eps_t = const.tile([P, 1], F32, tag="eps_t")
    nc.vector.memset(eps_t, EPS)

    bq_sb = const.tile([P, NPAIR], F32, tag="bq_sb")
    bk_sb = const.tile([P, NPAIR], F32, tag="bk_sb")
    b1_sb = const.tile([P, NF], F32, tag="b1_sb")
    nc.sync.dma_start(out=bq_sb, in_=_dram_ap(tn["bq"], 0, [[1, P], [P, NPAIR]]))
    nc.sync.dma_start(out=bk_sb, in_=_dram_ap(tn["bk"], 0, [[1, P], [P, NPAIR]]))
    nc.sync.dma_start(out=b1_sb, in_=_dram_ap(tn["b1"], 0, [[1, P], [P, NF]]))
    bv_bc = const.tile([P, H, KD], F32, tag="bv_bc")
    nc.sync.dma_start(out=bv_bc, in_=_dram_ap(tn["bv"], 0, [[0, P], [KD, H], [1, KD]]))
    b2_bc = const.tile([P, D], F32, tag="b2_bc")
    nc.sync.dma_start(out=b2_bc, in_=_dram_ap(tn["b2"], 0, [[0, P], [1, D]]))
    wo_aug = const.tile([KD + 1, H, KD], BF16, tag="wo_aug")
    nc.sync.dma_start(out=wo_aug[0:KD, :, :],
                      in_=_dram_ap(tn["wo"], 0, [[KD, KD], [KD * KD, H], [1, KD]]))
    nc.sync.dma_start(out=wo_aug[KD:KD + 1, :, :],
                      in_=_dram_ap(tn["bo"], 0, [[0, 1], [KD, H], [1, KD]]))
    # host-computed causal masks, one [P, 2, TL] dual tile per key tile st
    cmask = const.tile([P, NT, 2, TL], BF16, tag="cmask")
    for st in range(NT):
        nc.sync.dma_start(out=cmask[:, st, :, :], in_=_dram_ap(
            tn["cmask"], st * P * 2 * TL, [[2 * TL, P], [TL, 2], [1, TL]]))

    # ---------------- persistent activations ----------------
    h_sb = pers.tile([P, NTL, D], F32, tag="h_sb")
    xnT = pers_x.tile([P, ND, TL], BF16, tag="xnT")
    qT = pers.tile([P, NPAIR, TL], BF16, tag="qT")
    kT = pers.tile([P, NPAIR, T], BF16, tag="kT")
    v_full = pers.tile([P, NT, H, KD + 1], BF16, tag="v_full")
    klv = pers.tile([P, KVW], BF16, tag="klv")
    # scratch doubles as y16 (attention, [65, H, 512] slice) and aT_g (FFN)
    scratch = pers.tile([P, NF, 512], BF16, tag="scratch")
    L_sb = pers.tile([H, TL], BF16, tag="L_sb")
    linv = pers.tile([P, NTL, H], F32, tag="linv")

    nc.vector.memset(v_full[:, :, :, KD:KD + 1], 1.0)

    # ---------------- embedding ----------------
    for it in range(NTL):
        idx_t = small.tile([P, 1], mybir.dt.int32, tag="idx")
        nc.sync.dma_start(out=idx_t, in_=tn["x_idx"][it * P:(it + 1) * P, :])
        nc.gpsimd.indirect_dma_start(
            out=h_sb[:, it, :], out_offset=None, in_=tn["tok_emb"][:, :],
            in_offset=bass.IndirectOffsetOnAxis(ap=idx_t[:, :1], axis=0))
        pos_t = small.tile([P, D], F32, tag="pos", bufs=1)
        nc.sync.dma_start(out=pos_t, in_=tn["pos_emb"][it * P:(it + 1) * P, :])
        nc.vector.tensor_add(out=h_sb[:, it, :], in0=h_sb[:, it, :], in1=pos_t[:])

    # ---------------- helpers ----------------
    def layer_norm_T():
        """xnT = transpose(normalize(h_sb)) in bf16 (gamma/beta pre-folded)."""
        for it in range(NTL):
            stats = small.tile([P, 2, 6], F32, tag="bnst")
            mv = small.tile([P, 2], F32, tag="bnmv")
            for sg in range(2):
                nc.vector.bn_stats(out=stats[:, sg, :],
                                   in_=h_sb[:, it, sg * 512:(sg + 1) * 512])
            nc.vector.bn_aggr(out=mv, in_=stats)
            rstd = small.tile([P, 1], F32, tag="rstd")
            nc.scalar.activation(out=rstd, in_=mv[:, 1:2], func=AF.Sqrt,
                                 bias=eps_t[:, :], scale=1.0)
            nc.vector.reciprocal(out=rstd, in_=rstd)
            for half in range(2):
                xt4 = ev.tile([P, 512], BF16, tag="xt", bufs=3)
                nc.vector.tensor_scalar(
                    out=xt4, in0=h_sb[:, it, half * 512:(half + 1) * 512],
                    scalar1=mv[:, 0:1], scalar2=rstd,
                    op0=ALU.subtract, op1=ALU.mult)
                tp4 = ps.tile([P, 512], BF16, tag="work", bufs=2)
                for k in range(4):
                    nc.tensor.transpose(out=tp4[:, k * P:(k + 1) * P],
                                        in_=xt4[:, k * P:(k + 1) * P],
                                        identity=ident_bf[:])
                dst = xnT[:, half * 4:(half + 1) * 4, it * P:(it + 1) * P]
                src4 = tp4[:].rearrange("p (a b) -> p a b", a=4)
                if (it + half) % 2 == 0:
                    nc.scalar.copy(out=dst, in_=src4)
                else:
                    nc.vector.tensor_copy(dst, src4)

    # ---------------- transformer layers (tied weights) ----------------
    for _layer in range(LAYERS):
        layer_norm_T()

        # ---- k/v for local tokens -> klv staging ----
        for hp in range(NPAIR):
            wt = small.tile([P, ND, 2, KD], BF16, tag="wk", bufs=3)
            for hi in range(2):
                nc.sync.dma_start(out=wt[:, :, hi, :], in_=_dram_ap(
                    tn["wk"], (hp * 2 + hi) * D * KD,
                    [[KD, P], [P * KD, ND], [1, KD]]))
            pswt = ps.tile([P, 512], F32, tag="work", bufs=2)
            for idd in range(ND):
                nc.tensor.matmul(
                    out=pswt[:], lhsT=wt[:, idd, :, :], rhs=xnT[:, idd, :],
                    start=(idd == 0), stop=(idd == ND - 1))
            nc.scalar.activation(
                out=klv[:, hp * TL:(hp + 1) * TL], in_=pswt[:],
                func=AF.Identity, bias=bk_sb[:, hp:hp + 1], scale=1.0)
        for hc in range(2):
            wvt = small.tile([P, ND, 8, KD], BF16, tag="wv", bufs=1)
            for hh in range(8):
                nc.sync.dma_start(out=wvt[:, :, hh, :], in_=_dram_ap(
                    tn["wv"], (hc * 8 + hh) * D * KD,
                    [[KD, P], [P * KD, ND], [1, KD]]))
            for it in range(NTL):
                psv = ps.tile([P, 512], F32, tag="work", bufs=2)
                for idd in range(ND):
                    nc.tensor.matmul(
                        out=psv[:], lhsT=xnT[:, idd, it * P:(it + 1) * P],
                        rhs=wvt[:, idd, :, :].rearrange("p a b -> p (a b)"),
                        start=(idd == 0), stop=(idd == ND - 1))
                nc.vector.tensor_tensor(
                    out=klv[:, NPAIR * TL + it * H * KD + hc * 512:
                            NPAIR * TL + it * H * KD + (hc + 1) * 512]
                        .rearrange("p (a b) -> p a b", a=8),
                    in0=psv[:].rearrange("p (a b) -> p a b", a=8),
                    in1=bv_bc[:, hc * 8:(hc + 1) * 8, :], op=ALU.add)

        # ---- in-pair AllGather of k/v (DRAM bounce) ----
        kv_in = dram.tile([P, KVW], BF16, tag="kv_in")
        kv_out = dram.tile([2, P, KVW], BF16, tag="kv_out")
        nc.sync.dma_start(out=kv_in[:], in_=klv[:])
        nc.gpsimd.collective_compute(
            "AllGather", ALU.bypass, replica_groups=GROUPS,
            ins=[kv_in[:].opt()], outs=[kv_out[:].opt()])

        # ---- q for local tokens (overlaps the AllGather) ----
        for hp in range(NPAIR):
            wt = small.tile([P, ND, 2, KD], BF16, tag="wq", bufs=3)
            for hi in range(2):
                nc.sync.dma_start(out=wt[:, :, hi, :], in_=_dram_ap(
                    tn["wq"], (hp * 2 + hi) * D * KD,
                    [[KD, P], [P * KD, ND], [1, KD]]))
            pswt = ps.tile([P, 512], F32, tag="work", bufs=2)
            for idd in range(ND):
                nc.tensor.matmul(
                    out=pswt[:], lhsT=wt[:, idd, :, :], rhs=xnT[:, idd, :],
                    start=(idd == 0), stop=(idd == ND - 1))
            nc.scalar.activation(
                out=qT[:, hp, :], in_=pswt[:],
                func=AF.Identity, bias=bq_sb[:, hp:hp + 1], scale=1.0)

        # ---- gather full-T k/v from the AllGather result ----
        for rr in range(2):
            nc.sync.dma_start(
                out=kT[:, :, rr * TL:(rr + 1) * TL],
                in_=kv_out[rr, :, 0:NPAIR * TL]
                    .rearrange("p (a b) -> p a b", a=NPAIR))
            for itl in range(NTL):
                nc.sync.dma_start(
                    out=v_full[:, rr * NTL + itl, :, 0:KD],
                    in_=kv_out[rr, :, NPAIR * TL + itl * H * KD:
                               NPAIR * TL + (itl + 1) * H * KD]
                        .rearrange("p (a b) -> p a b", a=H))

        # ---- attention (local queries, full keys) ----
        for hp in range(NPAIR):
            y_ps = [ps.tile([KD + 1, 512], F32, tag="work", bufs=2,
                            name=f"y_ps{_hi}")
                    for _hi in range(2)]
            pts = {}

            def emit_scores(st, hp=hp, pts=pts):
                s_ps = ps.tile([P, 2, 512], F32, tag="dual", bufs=3,
                               name="s_ps")
                for hi in range(2):
                    nc.tensor.matmul(
                        out=s_ps[:, hi, :],
                        lhsT=kT[hi * KD:(hi + 1) * KD, hp, st * P:(st + 1) * P],
                        rhs=qT[hi * KD:(hi + 1) * KD, hp, :],
                        start=True, stop=True,
                        tile_position=((hi * KD, 0) if PACK_SCORES else None))
                pt = ev.tile([P, 2, 512], BF16, tag="pt", bufs=4, name="pt")
                nc.scalar.activation(out=pt[:], in_=s_ps[:], func=AF.Exp,
                                     scale=SCALE)
                nc.vector.tensor_tensor(
                    out=pt[:], in0=pt[:], in1=cmask[:, st, :, :], op=ALU.mult)
                pts[st] = pt

            def emit_av(st, hp=hp, y_ps=y_ps, pts=pts):
                pt = pts.pop(st)
                for hi in range(2):
                    nc.tensor.matmul(
                        out=y_ps[hi][:],
                        lhsT=v_full[:, st, 2 * hp + hi, :],
                        rhs=pt[:, hi, :],
                        start=(st == 0), stop=(st == NT - 1))

            for st in range(NT):
                emit_scores(st)
                if st >= 2:
                    emit_av(st - 2)
            emit_av(NT - 2)
            emit_av(NT - 1)
            for hi in range(2):
                h_ = 2 * hp + hi
                nc.vector.tensor_copy(scratch[0:KD + 1, h_, :], y_ps[hi][:])
                nc.sync.dma_start(out=L_sb[h_:h_ + 1, :],
                                  in_=scratch[KD:KD + 1, h_, :])
        # denominators -> per-token reciprocals
        for it in range(NTL):
            lt_ps = ps.tile([P, H], BF16, tag="work", bufs=2)
            nc.tensor.transpose(out=lt_ps[:], in_=L_sb[:, it * P:(it + 1) * P],
                                identity=ident_bf[0:H, 0:H])
            nc.vector.reciprocal(out=linv[:, it, :], in_=lt_ps[:])
        # o = concat_h(y_h @ Wo_h); h += o * linv (+ bo via ones row)
        for it in range(NTL):
            for half in range(2):
                o_ps = ps.tile([P, 512], F32, tag="work", bufs=2)
                for h8 in range(8):
                    h_ = half * 8 + h8
                    nc.tensor.matmul(
                        out=o_ps[:, h8 * KD:(h8 + 1) * KD],
                        lhsT=scratch[0:KD + 1, h_, it * P:(it + 1) * P],
                        rhs=wo_aug[:, h_, :],
                        start=(h8 == 0), stop=(h8 == 7))
                lap = linv[:, it, half * 8:(half + 1) * 8]
                lbc = bass.AP(tensor=lap.tensor, offset=lap.offset,
                              ap=[list(lap.ap[0]), list(lap.ap[-1]), [0, KD]])
                o_sb = ev.tile([P, 8, KD], F32, tag="o_sb", bufs=3)
                nc.vector.tensor_tensor(
                    out=o_sb[:], in0=o_ps[:].rearrange("p (a b) -> p a b", a=8),
                    in1=lbc, op=ALU.mult)
                nc.vector.tensor_tensor(
                    out=h_sb[:, it, half * 512:(half + 1) * 512],
                    in0=h_sb[:, it, half * 512:(half + 1) * 512],
                    in1=o_sb[:].rearrange("p a b -> p (a b)"), op=ALU.add)

        layer_norm_T()

        # ---- FFN (local 512 tokens) ----
        aT_g = scratch
        for fi in range(NF):
            w1t = small.tile([P, ND, P], BF16, tag="w1", bufs=4)
            nc.sync.dma_start(out=w1t, in_=_dram_ap(
                tn["w1"], fi * P, [[F, P], [P * F, ND], [1, P]]))
            a_ps = ps.tile([P, 512], F32, tag="work", bufs=2)
            for idd in range(ND):
                nc.tensor.matmul(
                    out=a_ps[:], lhsT=w1t[:, idd, :], rhs=xnT[:, idd, :],
                    start=(idd == 0), stop=(idd == ND - 1))
            nc.scalar.activation(
                out=aT_g[:, fi, :], in_=a_ps[:],
                func=AF.Relu, bias=b1_sb[:, fi:fi + 1], scale=1.0)
        for dc in range(2):
            ffp = [ps.tile([P, 2, 512], F32, tag="dual", bufs=3,
                           name=f"ffp{_pp}")
                   for _pp in range(2)]
            for fi in range(NF):
                w2t = small.tile([P, 512], BF16, tag="w2", bufs=4)
                nc.sync.dma_start(out=w2t, in_=_dram_ap(
                    tn["w2"], fi * P * D + dc * 512, [[D, P], [1, 512]]))
                for pair in range(2):
                    for ih in range(2):
                        nc.tensor.matmul(
                            out=ffp[pair][:, ih, :],
                            lhsT=aT_g[:, fi, (pair * 2 + ih) * P:
                                      (pair * 2 + ih + 1) * P],
                            rhs=w2t[:],
                            start=(fi == 0), stop=(fi == NF - 1))
            for pair in range(2):
                for ih in range(2):
                    it = pair * 2 + ih
                    nc.vector.tensor_tensor(
                        out=h_sb[:, it, dc * 512:(dc + 1) * 512],
                        in0=h_sb[:, it, dc * 512:(dc + 1) * 512],
                        in1=ffp[pair][:, ih, :], op=ALU.add)
        for it in range(NTL):
            nc.vector.tensor_add(out=h_sb[:, it, :], in0=h_sb[:, it, :],
                                 in1=b2_bc[:])

    # ---------------- final LN + vocab projection ----------------
    layer_norm_T()
    pers.release()
    wout_pool = tc.alloc_tile_pool(name="wout_pool", bufs=3)
    lg_pool = tc.alloc_tile_pool(name="lg_pool", bufs=6)
    for vc in range(NVC):
        bout_bc = small.tile([P, VC], F32, tag="bout", bufs=2)
        nc.sync.dma_start(out=bout_bc,
                          in_=_dram_ap(tn["bout"], vc * VC, [[0, P], [1, VC]]))
        wtl = wout_pool.tile([P, ND, VC], BF16, tag="wout")
        nc.sync.dma_start(out=wtl, in_=_dram_ap(
            tn["wout"], vc * VC, [[V, P], [P * V, ND], [1, VC]]))
        for it in range(NTL):
            lg_ps = ps.tile([P, VC], F32, tag="work", bufs=2)
            for idd in range(ND):
                nc.tensor.matmul(
                    out=lg_ps[:], lhsT=xnT[:, idd, it * P:(it + 1) * P],
                    rhs=wtl[:, idd, :],
                    start=(idd == 0), stop=(idd == ND - 1))
            lg_sb = lg_pool.tile([P, VC], F32, tag="lg")
            nc.vector.tensor_tensor(out=lg_sb[:], in0=lg_ps[:], in1=bout_bc[:],
                                    op=ALU.add)
            nc.sync.dma_start(
                out=_dram_ap(tn["logits"], it * P * V + vc * VC,
                             [[V, P], [1, VC]]),
                in_=lg_sb[:])
    lg_pool.release()
    wout_pool.release()
    ps.release()
    dram.release()
    ev.release()
    small.release()
    pers_x.release()
    const.release()


_PROGRAM = None


def _get_program():
    global _PROGRAM
    if _PROGRAM is None:
        _PROGRAM = build_program()
    return _PROGRAM


def make_in_maps(inputs):
    bf16 = mybir.dt.np(BF16)
    f = lambda k: np.asarray(inputs[k], dtype=np.float32)
    x = np.asarray(inputs["x"]).astype(np.int32)          # [B, T]

    ln1_g, ln1_b = f("ln1_g"), f("ln1_b")
    ln2_g, ln2_b = f("ln2_g"), f("ln2_b")
    lnf_g, lnf_b = f("lnf_g"), f("lnf_b")
    Wq, Wk, Wv = f("Wq"), f("Wk"), f("Wv")
    W1, Wout = f("W1"), f("Wout")

    # fold LN gamma into the consuming weights, beta into the biases
    wq_f = Wq * ln1_g[None, :, None]
    wk_f = Wk * ln1_g[None, :, None]
    wv_f = Wv * ln1_g[None, :, None]
    bq_f = f("bq") + np.einsum("d,hdk->hk", ln1_b, Wq)
    bk_f = f("bk") + np.einsum("d,hdk->hk", ln1_b, Wk)
    bv_f = f("bv") + np.einsum("d,hdk->hk", ln1_b, Wv)
    w1_f = W1 * ln2_g[:, None]
    b1_f = f("b1") + ln2_b @ W1
    wout_f = Wout * lnf_g[:, None]
    bout_f = (f("bout") + lnf_b @ Wout).reshape(1, V)

    c = np.ascontiguousarray
    pos_full = f("pos_emb")
    shared = {
        "tok_emb": c(f("tok_emb")),
        "wq": c(wq_f.astype(bf16)), "wk": c(wk_f.astype(bf16)),
        "wv": c(wv_f.astype(bf16)),
        "bq": c(bq_f), "bk": c(bk_f), "bv": c(bv_f),
        "wo": c(f("Wo").astype(bf16)), "bo": c(f("bo").astype(bf16)),
        "w1": c(w1_f.astype(bf16)), "b1": c(b1_f),
        "w2": c(f("W2").astype(bf16)), "b2": c(f("b2")),
        "wout": c(wout_f.astype(bf16)), "bout": c(bout_f),
    }
    # causal masks per pair half r: cmask[st, p, hi, j] = 1 iff
    # (r*TL + j) >= (st*P + p); identical for both heads hi of a pair
    cmasks = []
    for r in range(2):
        j = np.arange(TL)[None, :]
        p = np.arange(P)[:, None]
        m = np.zeros((NT, P, 2, TL), dtype=np.float32)
        for st in range(NT):
            allow = ((r * TL + j) >= (st * P + p)).astype(np.float32)
            m[st, :, 0, :] = allow
            m[st, :, 1, :] = allow
        cmasks.append(c(m.astype(bf16)))

    in_maps = []
    for cix in range(8):
        b, r = cix // 2, cix % 2
        m = dict(shared)
        m["x_idx"] = c(x[b, r * TL:(r + 1) * TL].reshape(TL, 1))
        m["pos_emb"] = c(pos_full[r * TL:(r + 1) * TL, :])
        m["cmask"] = cmasks[r]
        in_maps.append(m)
    return in_maps


def kernel(**inputs):
    in_maps = make_in_maps(inputs)
    nc = _get_program()
    res = run_bass_kernel_spmd(nc, in_maps, core_ids=list(range(8)))
    out = np.empty((B, T, V), dtype=np.float32)
    for cix in range(8):
        b, r = cix // 2, cix % 2
        out[b, r * TL:(r + 1) * TL, :] = res.results[cix]["logits"]
    return out


# revision 4
# speedup vs baseline: 1.2720x; 1.0596x over previous
"""Bass/Trainium2 kernel for nn_BigramLM (dense transformer, 8 NeuronCores), v2.

Sharding v4: sequence-parallel pairs. Core (2b + r) handles tokens
[r*512, (r+1)*512) of batch b for ALL heads/features: embeddings, LN, QKV,
FFN and the vocab projection (full V) are per-token and shard cleanly; for
attention each core produces k/v for its tokens, a per-layer in-pair
AllGather (2MB bf16 bounce via DRAM) assembles the full-T k/v, and causal
masks (host-computed per core, applied to every key tile) handle the
geometry so the SPMD program is identical on both pair members.

v2 changes vs baseline:
  - all matmul operands bf16 (psum stays fp32): fixes the fp32r LDWEIGHTS /
    stream-rate penalties seen in the trace (scores 655ns -> ~220ns etc).
  - LN gamma/beta folded into the following weights/biases host-side, so
    on-device LN is stats + normalize + transpose only.
  - QKV/attention activations stay in SBUF (no DRAM scratch round-trips).
  - scores computed per head-pair with row-group packed matmuls
    (tile_position (0,0)/(64,0)), exp over [128,1024] dual tiles.
  - causal masks applied on GpSimd (otherwise idle), freeing DVE.
  - attention o = concat_h(y_h @ Wo_h) accumulated column-wise in one psum
    bank per (it, half); softmax normalization folded after Wo via the
    denominator row produced by the augmented-V ones column.
  - FFN accumulates all 32 f-tiles into resident psum, evacuating once.
"""

import sys

sys.path.insert(0, "/opt/trn_rl_repo")

import numpy as np

import concourse.bass as bass
import concourse.mybir as mybir
import concourse.tile as tile
from concourse import bacc
from concourse.bass_utils import run_bass_kernel_spmd
from concourse.masks import make_identity

F32 = mybir.dt.float32
BF16 = mybir.dt.bfloat16
AF = mybir.ActivationFunctionType
ALU = mybir.AluOpType

V, D, H, KD, B, T = 32000, 1024, 16, 64, 4, 1024
F = 4 * D
LAYERS = 4
P = 128
TL = 512               # local tokens per core
NT = T // P            # 8 key tiles (full T)
NTL = TL // P          # 4 local token tiles
ND = D // P            # 8 d tiles
NF = F // P            # 32 f tiles
NPAIR = H // 2         # 8 head pairs
KVW = NPAIR * TL + NTL * H * KD   # 8192: flat k|v bounce width per core
VC = 500               # vocab chunk (psum free dim)
NVC = V // VC          # 64 (full vocab per core)
EPS = 1e-5
SCALE = 1.0 / float(np.sqrt(KD))

PACK_SCORES = True     # row-group packed dual-head score matmuls


def _dram_ap(handle, offset, pattern):
    t = getattr(handle, "tensor", handle)
    offset = offset + getattr(handle, "offset", 0)
    return bass.AP(tensor=t, offset=offset, ap=[list(p) for p in pattern])


def build_program():
    nc = bacc.Bacc("TRN2", target_bir_lowering=False, debug=False, num_devices=8)

    tn = {}
    tn["x_idx"] = nc.dram_tensor("x_idx", [TL, 1], mybir.dt.int32, kind="ExternalInput")
    tn["tok_emb"] = nc.dram_tensor("tok_emb", [V, D], F32, kind="ExternalInput")
    tn["pos_emb"] = nc.dram_tensor("pos_emb", [TL, D], F32, kind="ExternalInput")
    tn["cmask"] = nc.dram_tensor("cmask", [NT, P, 2, TL], BF16, kind="ExternalInput")
    for nm, shp, dt in (
        ("wq", [H, D, KD], BF16), ("wk", [H, D, KD], BF16), ("wv", [H, D, KD], BF16),
        ("bq", [H, KD], F32), ("bk", [H, KD], F32), ("bv", [H, KD], F32),
        ("wo", [H, KD, KD], BF16), ("bo", [H, KD], BF16),
        ("w1", [D, F], BF16), ("b1", [F], F32),
        ("w2", [F, D], BF16), ("b2", [D], F32),
        ("wout", [D, V], BF16), ("bout", [1, V], F32),
    ):
        tn[nm] = nc.dram_tensor(nm, shp, dt, kind="ExternalInput")
    tn["logits"] = nc.dram_tensor("logits", [TL, V], BF16, kind="ExternalOutput")

    with tile.TileContext(nc) as tc:
        _body(nc, tc, tn)
    nc.compile()
    return nc


GROUPS = [[0, 1], [2, 3], [4, 5], [6, 7]]


def _body(nc, tc, tn):
    const = tc.alloc_tile_pool(name="const", bufs=1)
    pers_x = tc.alloc_tile_pool(name="pers_x", bufs=1)   # xnT: lives to the end
    small = tc.alloc_tile_pool(name="small", bufs=3)
    ev = tc.alloc_tile_pool(name="ev", bufs=2)
    ps = tc.alloc_tile_pool(name="ps", bufs=1, space="PSUM")
    dram = tc.alloc_tile_pool(name="dram", bufs=2, space="DRAM")
    # top of the SBUF pool stack: released before the vocab projection
    pers = tc.alloc_tile_pool(name="pers", bufs=1)

    # ---------------- constants ----------------
    ident_bf = const.tile([P, P], BF16, tag="ident_bf")
    make_identity(nc, ident_bf)
    eps_t = const.tile([P, 1], F32, tag="eps_t")
    nc.vector.memset(eps_t, EPS)

    bq_sb = const.tile([P, NPAIR], F32, tag="bq_sb")
    bk_sb = const.tile([P, NPAIR], F32, tag="bk_sb")
    b1_sb = const.tile([P, NF], F32, tag="b1_sb")
    nc.sync.dma_start(out=bq_sb, in_=_dram_ap(tn["bq"], 0, [[1, P], [P, NPAIR]]))
    nc.sync.dma_start(out=bk_sb, in_=_dram_ap(tn["bk"], 0, [[1, P], [P, NPAIR]]))
    nc.sync.dma_start(out=b1_sb, in_=_dram_ap(tn["b1"], 0, [[1, P], [P, NF]]))
    bv_bc = const.tile([P, H, KD], F32, tag="bv_bc")
    nc.sync.dma_start(out=bv_bc, in_=_dram_ap(tn["bv"], 0, [[0, P], [KD, H], [1, KD]]))
    b2_bc = const.tile([P, D], F32, tag="b2_bc")
    nc.sync.dma_start(out=b2_bc, in_=_dram_ap(tn["b2"], 0, [[0, P], [1, D]]))
    wo_aug = const.tile([KD + 1, H, KD], BF16, tag="wo_aug")
    nc.sync.dma_start(out=wo_aug[0:KD, :, :],
                      in_=_dram_ap(tn["wo"], 0, [[KD, KD], [KD * KD, H], [1, KD]]))
    nc.sync.dma_start(out=wo_aug[KD:KD + 1, :, :],
                      in_=_dram_ap(tn["bo"], 0, [[0, 1], [KD, H], [1, KD]]))
    # host-computed causal masks, one [P, 2, TL] dual tile per key tile st
    cmask = const.tile([P, NT, 2, TL], BF16, tag="cmask")
    for st in range(NT):
        nc.sync.dma_start(out=cmask[:, st, :, :], in_=_dram_ap(
            tn["cmask"], st * P * 2 * TL, [[2 * TL, P], [TL, 2], [1, TL]]))

    # ---------------- persistent activations ----------------
    h_sb = pers.tile([P, NTL, D], F32, tag="h_sb")
    xnT = pers_x.tile([P, ND, TL], BF16, tag="xnT")
    qT = pers.tile([P, NPAIR, TL], BF16, tag="qT")
    kT = pers.tile([P, NPAIR, T], BF16, tag="kT")
    v_full = pers.tile([P, NT, H, KD + 1], BF16, tag="v_full")
    klv = pers.tile([P, KVW], BF16, tag="klv")
    # scratch doubles as y16 (attention, [65, H, 512] slice) and aT_g (FFN)
    scratch = pers.tile([P, NF, 512], BF16, tag="scratch")
    L_sb = pers.tile([H, TL], BF16, tag="L_sb")
    linv = pers.tile([P, NTL, H], F32, tag="linv")

    nc.vector.memset(v_full[:, :, :, KD:KD + 1], 1.0)

    # ---------------- embedding ----------------
    for it in range(NTL):
        idx_t = small.tile([P, 1], mybir.dt.int32, tag="idx")
        nc.sync.dma_start(out=idx_t, in_=tn["x_idx"][it * P:(it + 1) * P, :])
        nc.gpsimd.indirect_dma_start(
            out=h_sb[:, it, :], out_offset=None, in_=tn["tok_emb"][:, :],
            in_offset=bass.IndirectOffsetOnAxis(ap=idx_t[:, :1], axis=0))
        pos_t = small.tile([P, D], F32, tag="pos", bufs=1)
        nc.sync.dma_start(out=pos_t, in_=tn["pos_emb"][it * P:(it + 1) * P, :])
        nc.vector.tensor_add(out=h_sb[:, it, :], in0=h_sb[:, it, :], in1=pos_t[:])

    # ---------------- helpers ----------------
    def layer_norm_T():
        """xnT = transpose(normalize(h_sb)) in bf16 (gamma/beta pre-folded)."""
        for it in range(NTL):
            stats = small.tile([P, 2, 6], F32, tag="bnst")
            mv = small.tile([P, 2], F32, tag="bnmv")
            for sg in range(2):
                nc.vector.bn_stats(out=stats[:, sg, :],
                                   in_=h_sb[:, it, sg * 512:(sg + 1) * 512])
            nc.vector.bn_aggr(out=mv, in_=stats)
            rstd = small.tile([P, 1], F32, tag="rstd")
            nc.scalar.activation(out=rstd, in_=mv[:, 1:2], func=AF.Sqrt,
                                 bias=eps_t[:, :], scale=1.0)
            nc.vector.reciprocal(out=rstd, in_=rstd)
            for half in range(2):
                xt4 = ev.tile([P, 512], BF16, tag="xt", bufs=3)
                nc.vector.tensor_scalar(
                    out=xt4, in0=h_sb[:, it, half * 512:(half + 1) * 512],
                    scalar1=mv[:, 0:1], scalar2=rstd,
                    op0=ALU.subtract, op1=ALU.mult)
                tp4 = ps.tile([P, 512], BF16, tag="work", bufs=2)
                for k in range(4):
                    nc.tensor.transpose(out=tp4[:, k * P:(k + 1) * P],
                                        in_=xt4[:, k * P:(k + 1) * P],
                                        identity=ident_bf[:])
                dst = xnT[:, half * 4:(half + 1) * 4, it * P:(it + 1) * P]
                src4 = tp4[:].rearrange("p (a b) -> p a b", a=4)
                if (it + half) % 2 == 0:
                    nc.scalar.copy(out=dst, in_=src4)
                else:
                    nc.vector.tensor_copy(dst, src4)

    # ---------------- transformer layers (tied weights) ----------------
    for _layer in range(LAYERS):
        layer_norm_T()

        # ---- k/v for local tokens -> klv staging ----
        for hp in range(NPAIR):
            wt = small.tile([P, ND, 2, KD], BF16, tag="wk", bufs=3)
            for hi in range(2):
                nc.sync.dma_start(out=wt[:, :, hi, :], in_=_dram_ap(
                    tn["wk"], (hp * 2 + hi) * D * KD,
                    [[KD, P], [P * KD, ND], [1, KD]]))
            pswt = ps.tile([P, 512], F32, tag="work", bufs=2)
            for idd in range(ND):
                nc.tensor.matmul(
                    out=pswt[:], lhsT=wt[:, idd, :, :], rhs=xnT[:, idd, :],
                    start=(idd == 0), stop=(idd == ND - 1))
            nc.scalar.activation(
                out=klv[:, hp * TL:(hp + 1) * TL], in_=pswt[:],
                func=AF.Identity, bias=bk_sb[:, hp:hp + 1], scale=1.0)
        for hc in range(2):
            wvt = small.tile([P, ND, 8, KD], BF16, tag="wv", bufs=1)
            for hh in range(8):
                nc.sync.dma_start(out=wvt[:, :, hh, :], in_=_dram_ap(
                    tn["wv"], (hc * 8 + hh) * D * KD,
                    [[KD, P], [P * KD, ND], [1, KD]]))
            for it in range(NTL):
                psv = ps.tile([P, 512], F32, tag="work", bufs=2)
                for idd in range(ND):
                    nc.tensor.matmul(
                        out=psv[:], lhsT=xnT[:, idd, it * P:(it + 1) * P],
                        rhs=wvt[:, idd, :, :].rearrange("p a b -> p (a b)"),
                        start=(idd == 0), stop=(idd == ND - 1))
                nc.vector.tensor_tensor(
                    out=klv[:, NPAIR * TL + it * H * KD + hc * 512:
                            NPAIR * TL + it * H * KD + (hc + 1) * 512]
                        .rearrange("p (a b) -> p a b", a=8),
                    in0=psv[:].rearrange("p (a b) -> p a b", a=8),
                    in1=bv_bc[:, hc * 8:(hc + 1) * 8, :], op=ALU.add)

        # ---- in-pair AllGather of k (hidden under v/q projections) ----
        k_in = dram.tile([P, NPAIR * TL], BF16, tag="k_in")
        k_out = dram.tile([2, P, NPAIR * TL], BF16, tag="k_out")
        nc.sync.dma_start(out=k_in[:], in_=klv[:, 0:NPAIR * TL])
        nc.gpsimd.collective_compute(
            "AllGather", ALU.bypass, replica_groups=GROUPS,
            ins=[k_in[:].opt()], outs=[k_out[:].opt()])

        # ---- q for local tokens (overlaps the AllGather) ----
        for hp in range(NPAIR):
            wt = small.tile([P, ND, 2, KD], BF16, tag="wq", bufs=3)
            for hi in range(2):
                nc.sync.dma_start(out=wt[:, :, hi, :], in_=_dram_ap(
                    tn["wq"], (hp * 2 + hi) * D * KD,
                    [[KD, P], [P * KD, ND], [1, KD]]))
            pswt = ps.tile([P, 512], F32, tag="work", bufs=2)
            for idd in range(ND):
                nc.tensor.matmul(
                    out=pswt[:], lhsT=wt[:, idd, :, :], rhs=xnT[:, idd, :],
                    start=(idd == 0), stop=(idd == ND - 1))
            nc.scalar.activation(
                out=qT[:, hp, :], in_=pswt[:],
                func=AF.Identity, bias=bq_sb[:, hp:hp + 1], scale=1.0)

        # ---- AllGather of v; gather full-T k/v from the results ----
        v_in = dram.tile([P, NTL * H * KD], BF16, tag="v_in")
        v_out = dram.tile([2, P, NTL * H * KD], BF16, tag="v_out")
        nc.sync.dma_start(out=v_in[:], in_=klv[:, NPAIR * TL:KVW])
        nc.gpsimd.collective_compute(
            "AllGather", ALU.bypass, replica_groups=GROUPS,
            ins=[v_in[:].opt()], outs=[v_out[:].opt()])
        for rr in range(2):
            nc.sync.dma_start(
                out=kT[:, :, rr * TL:(rr + 1) * TL],
                in_=k_out[rr, :, :].rearrange("p (a b) -> p a b", a=NPAIR))
            for itl in range(NTL):
                nc.sync.dma_start(
                    out=v_full[:, rr * NTL + itl, :, 0:KD],
                    in_=v_out[rr, :, itl * H * KD:(itl + 1) * H * KD]
                        .rearrange("p (a b) -> p a b", a=H))

        # ---- attention (local queries, full keys) ----
        for hp in range(NPAIR):
            y_ps = [ps.tile([KD + 1, 512], F32, tag="work", bufs=2,
                            name=f"y_ps{_hi}")
                    for _hi in range(2)]
            pts = {}

            def emit_scores(st, hp=hp, pts=pts):
                s_ps = ps.tile([P, 2, 512], F32, tag="dual", bufs=3,
                               name="s_ps")
                for hi in range(2):
                    nc.tensor.matmul(
                        out=s_ps[:, hi, :],
                        lhsT=kT[hi * KD:(hi + 1) * KD, hp, st * P:(st + 1) * P],
                        rhs=qT[hi * KD:(hi + 1) * KD, hp, :],
                        start=True, stop=True,
                        tile_position=((hi * KD, 0) if PACK_SCORES else None))
                pt = ev.tile([P, 2, 512], BF16, tag="pt", bufs=4, name="pt")
                nc.scalar.activation(out=pt[:], in_=s_ps[:], func=AF.Exp,
                                     scale=SCALE)
                nc.vector.tensor_tensor(
                    out=pt[:], in0=pt[:], in1=cmask[:, st, :, :], op=ALU.mult)
                pts[st] = pt

            def emit_av(st, hp=hp, y_ps=y_ps, pts=pts):
                pt = pts.pop(st)
                for hi in range(2):
                    nc.tensor.matmul(
                        out=y_ps[hi][:],
                        lhsT=v_full[:, st, 2 * hp + hi, :],
                        rhs=pt[:, hi, :],
                        start=(st == 0), stop=(st == NT - 1))

            for st in range(NT):
                emit_scores(st)
                if st >= 2:
                    emit_av(st - 2)
            emit_av(NT - 2)
            emit_av(NT - 1)
            for hi in range(2):
                h_ = 2 * hp + hi
                nc.vector.tensor_copy(scratch[0:KD + 1, h_, :], y_ps[hi][:])
                nc.sync.dma_start(out=L_sb[h_:h_ + 1, :],
                                  in_=scratch[KD:KD + 1, h_, :])
        # denominators -> per-token reciprocals
        for it in range(NTL):
            lt_ps = ps.tile([P, H], BF16, tag="work", bufs=2)
            nc.tensor.transpose(out=lt_ps[:], in_=L_sb[:, it * P:(it + 1) * P],
                                identity=ident_bf[0:H, 0:H])
            nc.vector.reciprocal(out=linv[:, it, :], in_=lt_ps[:])
        # o = concat_h(y_h @ Wo_h); h += o * linv (+ bo via ones row)
        for it in range(NTL):
            for half in range(2):
                o_ps = ps.tile([P, 512], F32, tag="work", bufs=2)
                for h8 in range(8):
                    h_ = half * 8 + h8
                    nc.tensor.matmul(
                        out=o_ps[:, h8 * KD:(h8 + 1) * KD],
                        lhsT=scratch[0:KD + 1, h_, it * P:(it + 1) * P],
                        rhs=wo_aug[:, h_, :],
                        start=(h8 == 0), stop=(h8 == 7))
                lap = linv[:, it, half * 8:(half + 1) * 8]
                lbc = bass.AP(tensor=lap.tensor, offset=lap.offset,
                              ap=[list(lap.ap[0]), list(lap.ap[-1]), [0, KD]])
                o_sb = ev.tile([P, 8, KD], F32, tag="o_sb", bufs=3)
                nc.vector.tensor_tensor(
                    out=o_sb[:], in0=o_ps[:].rearrange("p (a b) -> p a b", a=8),
                    in1=lbc, op=ALU.mult)
                nc.vector.tensor_tensor(
                    out=h_sb[:, it, half * 512:(half + 1) * 512],
                    in0=h_sb[:, it, half * 512:(half + 1) * 512],
                    in1=o_sb[:].rearrange("p a b -> p (a b)"), op=ALU.add)

        layer_norm_T()

        # ---- FFN (local 512 tokens) ----
        aT_g = scratch
        for fi in range(NF):
            w1t = small.tile([P, ND, P], BF16, tag="w1", bufs=4)
            nc.sync.dma_start(out=w1t, in_=_dram_ap(
                tn["w1"], fi * P, [[F, P], [P * F, ND], [1, P]]))
            a_ps = ps.tile([P, 512], F32, tag="work", bufs=2)
            for idd in range(ND):
                nc.tensor.matmul(
                    out=a_ps[:], lhsT=w1t[:, idd, :], rhs=xnT[:, idd, :],
                    start=(idd == 0), stop=(idd == ND - 1))
            nc.scalar.activation(
                out=aT_g[:, fi, :], in_=a_ps[:],
                func=AF.Relu, bias=b1_sb[:, fi:fi + 1], scale=1.0)
        for dc in range(2):
            ffp = [ps.tile([P, 2, 512], F32, tag="dual", bufs=3,
                           name=f"ffp{_pp}")
                   for _pp in range(2)]
            for fi in range(NF):
                w2t = small.tile([P, 512], BF16, tag="w2", bufs=4)
                nc.sync.dma_start(out=w2t, in_=_dram_ap(
                    tn["w2"], fi * P * D + dc * 512, [[D, P], [1, 512]]))
                for pair in range(2):
                    for ih in range(2):
                        nc.tensor.matmul(
                            out=ffp[pair][:, ih, :],
                            lhsT=aT_g[:, fi, (pair * 2 + ih) * P:
                                      (pair * 2 + ih + 1) * P],
                            rhs=w2t[:],
                            start=(fi == 0), stop=(fi == NF - 1))
            for pair in range(2):
                for ih in range(2):
                    it = pair * 2 + ih
                    nc.vector.tensor_tensor(
                        out=h_sb[:, it, dc * 512:(dc + 1) * 512],
                        in0=h_sb[:, it, dc * 512:(dc + 1) * 512],
                        in1=ffp[pair][:, ih, :], op=ALU.add)
        for it in range(NTL):
            nc.vector.tensor_add(out=h_sb[:, it, :], in0=h_sb[:, it, :],
                                 in1=b2_bc[:])

    # ---------------- final LN + vocab projection ----------------
    layer_norm_T()
    pers.release()
    wout_pool = tc.alloc_tile_pool(name="wout_pool", bufs=4)
    lg_pool = tc.alloc_tile_pool(name="lg_pool", bufs=6)
    for vc in range(NVC):
        bout_bc = small.tile([P, VC], F32, tag="bout", bufs=2)
        nc.sync.dma_start(out=bout_bc,
                          in_=_dram_ap(tn["bout"], vc * VC, [[0, P], [1, VC]]))
        wtl = wout_pool.tile([P, ND, VC], BF16, tag="wout")
        nc.sync.dma_start(out=wtl, in_=_dram_ap(
            tn["wout"], vc * VC, [[V, P], [P * V, ND], [1, VC]]))
        for it in range(NTL):
            lg_ps = ps.tile([P, VC], F32, tag=("work" if it % 2 == 0 else "dual"),
                            bufs=(2 if it % 2 == 0 else 3))
            for idd in range(ND):
                nc.tensor.matmul(
                    out=lg_ps[:], lhsT=xnT[:, idd, it * P:(it + 1) * P],
                    rhs=wtl[:, idd, :],
                    start=(idd == 0), stop=(idd == ND - 1))
            lg_sb = lg_pool.tile([P, VC], BF16, tag="lg")
            nc.vector.tensor_tensor(out=lg_sb[:], in0=lg_ps[:], in1=bout_bc[:],
                                    op=ALU.add)
            nc.sync.dma_start(
                out=_dram_ap(tn["logits"], it * P * V + vc * VC,
                             [[V, P], [1, VC]]),
                in_=lg_sb[:])
    lg_pool.release()
    wout_pool.release()
    ps.release()
    dram.release()
    ev.release()
    small.release()
    pers_x.release()
    const.release()


_PROGRAM = None


def _get_program():
    global _PROGRAM
    if _PROGRAM is None:
        _PROGRAM = build_program()
    return _PROGRAM


def make_in_maps(inputs):
    bf16 = mybir.dt.np(BF16)
    f = lambda k: np.asarray(inputs[k], dtype=np.float32)
    x = np.asarray(inputs["x"]).astype(np.int32)          # [B, T]

    ln1_g, ln1_b = f("ln1_g"), f("ln1_b")
    ln2_g, ln2_b = f("ln2_g"), f("ln2_b")
    lnf_g, lnf_b = f("lnf_g"), f("lnf_b")
    Wq, Wk, Wv = f("Wq"), f("Wk"), f("Wv")
    W1, Wout = f("W1"), f("Wout")

    # fold LN gamma into the consuming weights, beta into the biases
    wq_f = Wq * ln1_g[None, :, None]
    wk_f = Wk * ln1_g[None, :, None]
    wv_f = Wv * ln1_g[None, :, None]
    bq_f = f("bq") + np.einsum("d,hdk->hk", ln1_b, Wq)
    bk_f = f("bk") + np.einsum("d,hdk->hk", ln1_b, Wk)
    bv_f = f("bv") + np.einsum("d,hdk->hk", ln1_b, Wv)
    w1_f = W1 * ln2_g[:, None]
    b1_f = f("b1") + ln2_b @ W1
    wout_f = Wout * lnf_g[:, None]
    bout_f = (f("bout") + lnf_b @ Wout).reshape(1, V)

    c = np.ascontiguousarray
    pos_full = f("pos_emb")
    shared = {
        "tok_emb": c(f("tok_emb")),
        "wq": c(wq_f.astype(bf16)), "wk": c(wk_f.astype(bf16)),
        "wv": c(wv_f.astype(bf16)),
        "bq": c(bq_f), "bk": c(bk_f), "bv": c(bv_f),
        "wo": c(f("Wo").astype(bf16)), "bo": c(f("bo").astype(bf16)),
        "w1": c(w1_f.astype(bf16)), "b1": c(b1_f),
        "w2": c(f("W2").astype(bf16)), "b2": c(f("b2")),
        "wout": c(wout_f.astype(bf16)), "bout": c(bout_f),
    }
    # causal masks per pair half r: cmask[st, p, hi, j] = 1 iff
    # (r*TL + j) >= (st*P + p); identical for both heads hi of a pair
    cmasks = []
    for r in range(2):
        j = np.arange(TL)[None, :]
        p = np.arange(P)[:, None]
        m = np.zeros((NT, P, 2, TL), dtype=np.float32)
        for st in range(NT):
            allow = ((r * TL + j) >= (st * P + p)).astype(np.float32)
            m[st, :, 0, :] = allow
            m[st, :, 1, :] = allow
        cmasks.append(c(m.astype(bf16)))

    in_maps = []
    for cix in range(8):
        b, r = cix // 2, cix % 2
        m = dict(shared)
        m["x_idx"] = c(x[b, r * TL:(r + 1) * TL].reshape(TL, 1))
        m["pos_emb"] = c(pos_full[r * TL:(r + 1) * TL, :])
        m["cmask"] = cmasks[r]
        in_maps.append(m)
    return in_maps


def kernel(**inputs):
    in_maps = make_in_maps(inputs)
    nc = _get_program()
    res = run_bass_kernel_spmd(nc, in_maps, core_ids=list(range(8)))
    out = np.empty((B, T, V), dtype=np.float32)
    for cix in range(8):
        b, r = cix // 2, cix % 2
        out[b, r * TL:(r + 1) * TL, :] = (
            res.results[cix]["logits"].astype(np.float32))
    return out
